# revision 30
# baseline (speedup 1.0000x reference)
"""DigitCaps (capsule routing) Trainium2 Bass kernel.

u [512, 1152, 8] f32, W [1, 1152, 10, 16, 8] f32 -> v [512, 10, 16] f32
(3 dynamic-routing iterations, softmax over 10 classes).

Pure data-parallel: batch 64 per core x 8 cores; everything on-chip;
u_hat (377MB) is never materialized. Per routing iteration:
  T[b,i,c,k] = sum_d W[i,c,d,k] v[b,c,d]     PE (lhsT = W rows (c2,d16),
                                              rhs = block-diag v^T;
                                              output i-major)
  L[b,i,c]  += sum_k u[b,i,k] T[b,i,c,k]     DVE mul + tree adds (bf16)
  cexp       = exp(L) (ACT); den/rec via DVE adds+recip (all i-major)
  x_c        = cexp_c * (u * recT)           DVE (per class)
  s[b,c,:]   = sum_{ik} W x_c                PE (72 accumulating matmuls)
  v          = squash(s)
Everything softmax/logit-related lives i-major, so no per-iteration
layout transposes are needed anywhere.

Layouts (per core, B=64):
  i: block g = i//128 (9 blocks), partition r = i%128
  class c = 2p+ch, pass p in [0,5), parity ch in {0,1}
  logits/exp: [r, p, (g, ch, b)]
"""

import numpy as np

N_CORES = 8
B_PER = 64
I_CAPS = 1152
K_DIM = 8
C_CLS = 10
D_DIM = 16
NG = I_CAPS // 128  # 9
EPS = 1e-8

_CACHE = {}


def _build():
    import concourse.bass as bass
    import concourse.mybir as mybir
    from concourse import tile, bacc

    f32 = mybir.dt.float32
    bf16 = mybir.dt.bfloat16
    AF = mybir.ActivationFunctionType
    OP = mybir.AluOpType

    nc = bacc.Bacc()
    u_in = nc.dram_tensor("u", [B_PER, I_CAPS, K_DIM], f32, kind="ExternalInput")
    w_in = nc.dram_tensor("w", [I_CAPS, C_CLS, D_DIM, K_DIM], f32, kind="ExternalInput")
    eye128 = nc.dram_tensor("eye128", [128, 128], f32, kind="ExternalInput")
    v_out = nc.dram_tensor("v", [B_PER, C_CLS, D_DIM], f32, kind="ExternalOutput")

    with tile.TileContext(nc) as tc:
        perm = tc.alloc_tile_pool(name="perm", bufs=1)
        Wsk = perm.tile([128, K_DIM, NG, C_CLS, D_DIM], bf16)  # [r,(k,g,c,d)]
        WT = perm.tile([128, K_DIM, I_CAPS], bf16)   # rows 16c+d (classes 0-7)
        WTB = perm.tile([128, K_DIM, I_CAPS], bf16)  # rows 16(c-2)+d; 96:128 used
        uTk = perm.tile([128, K_DIM, NG, B_PER], bf16)      # u[b, 128g+r, k]
        L = perm.tile([128, 5, NG, 2, B_PER], bf16, name="Lt")    # logits i-major
        cE = perm.tile([128, 5, NG, 2, B_PER], bf16, name="cEt")  # exp(L)
        recT = perm.tile([128, NG, B_PER], bf16, name="recTt")    # 1/den i-major
        vT = perm.tile([128, 128], bf16)             # block-diag v^T classes 0-7
        vT4 = perm.tile([128, 128], bf16)            # rows 96:128: classes 8,9
        v_sb = perm.tile([64, C_CLS, D_DIM], f32, name="vsbt")
        s_sb = perm.tile([64, C_CLS, D_DIM], f32, name="ssbt")
        eye_sb = perm.tile([128, 128], f32)
        in2 = perm.tile([128, 128], f32)
        in2b = perm.tile([128, 128], f32)
        sq = perm.tile([64, C_CLS, D_DIM], f32)
        n2 = perm.tile([64, C_CLS], f32)
        t1 = perm.tile([64, C_CLS], f32)
        r1 = perm.tile([64, C_CLS], f32)
        f1 = perm.tile([64, C_CLS], f32)
        nrm = perm.tile([64, C_CLS], f32)
        nrm2 = perm.tile([64, C_CLS], f32)
        r2 = perm.tile([64, C_CLS], f32)
        fac = perm.tile([64, C_CLS], f32)

        nc.sync.dma_start(eye_sb[:], eye128[:])

        psS = tc.alloc_tile_pool(name="psS", bufs=2, space="PSUM")
        psT = tc.alloc_tile_pool(name="psT", bufs=2, space="PSUM")

        # ---------------- setup ----------------
        setup = tc.alloc_tile_pool(name="setup", bufs=1)
        u_stage = setup.tile([64, I_CAPS * K_DIM], f32, tag="ustage")
        nc.sync.dma_start(u_stage[:], u_in.rearrange("b i k -> b (i k)"))
        u_kmaj = u_stage[:].rearrange("b (i k) -> b k i", k=K_DIM)
        # uTk via PE transposes of u (f32, strided) blocks [64,128] -> [128,64]
        for k in range(K_DIM):
            for g in range(NG):
                ptr = psT.tile([128, I_CAPS], f32, tag="pt")
                nc.tensor.transpose(
                    ptr[:, 0:64],
                    u_kmaj[:, k, 128 * g : 128 * (g + 1)],
                    eye_sb[0:64, 0:64],
                )
                nc.scalar.copy(uTk[:, k, g, :], ptr[:, 0:64])
        # Wsk: contiguous row-block DMAs + strided cast-rearrange
        for g in range(NG):
            w_stage = setup.tile(
                [128, C_CLS * D_DIM * K_DIM], f32, tag="wstage", bufs=2
            )
            nc.sync.dma_start(
                w_stage[:],
                w_in[128 * g : 128 * (g + 1), :, :, :].rearrange(
                    "i c d k -> i (c d k)"
                ),
            )
            nc.gpsimd.tensor_copy(
                Wsk[:, :, g, :, :],
                w_stage[:].rearrange("i (c d k) -> i k c d", c=C_CLS, d=D_DIM),
            )
        # WT rows 16c+d (classes 0-7); WTB rows 16(c-2)+d (classes 8,9 at
        # rows 96:128): stage a contiguous [128, (k g 128)] copy, then one
        # batched xbar transpose (out 3D: [128, 72, 128])
        for (dst, c0) in ((WT, 0), (WTB, 2)):
            wtstg = setup.tile([128, K_DIM * NG * 128], bf16, tag="wtstg")
            nc.vector.tensor_copy(
                wtstg[:].rearrange("r (k g q) -> r k g q", k=K_DIM, g=NG),
                Wsk[:].rearrange("r k g c d -> r k g (c d)")[
                    :, :, :, 16 * c0 : 16 * c0 + 128
                ],
            )
            nc.sync.dma_start_transpose(
                dst[:].rearrange("r k (g q) -> r (k g) q", q=128),
                wtstg[:],
            )
        setup.release()

        nc.vector.memset(in2[:], 0.0)
        nc.vector.memset(in2b[:], 0.0)
        nc.gpsimd.memset(L[:], 0.0)

        itp = tc.alloc_tile_pool(name="itp", bufs=2)
        smp = tc.alloc_tile_pool(name="smp", bufs=3)

        def s_phase_s0():
            ps = psS.tile([64, C_CLS * D_DIM], f32, tag="ps_s")
            n = 0
            for k in range(K_DIM):
                for g in range(NG):
                    nc.tensor.matmul(
                        ps[:],
                        uTk[:, k, g, :],
                        Wsk[:, k, g, :, :].rearrange("r c d -> r (c d)"),
                        start=(n == 0),
                        stop=(n == K_DIM * NG - 1),
                    )
                    n += 1
            nc.scalar.activation(
                s_sb[:].rearrange("b c d -> b (c d)"), ps[:], AF.Copy, scale=0.1
            )

        def squash():
            nc.scalar.square(sq[:], s_sb[:])
            nc.vector.reduce_sum(n2[:], sq[:], axis=mybir.AxisListType.X)
            nc.scalar.add(t1[:], n2[:], 1.0)
            nc.vector.reciprocal(r1[:], t1[:])
            nc.vector.tensor_mul(f1[:], n2[:], r1[:])
            nc.scalar.sqrt(nrm[:], n2[:])
            nc.vector.tensor_scalar_add(nrm2[:], nrm[:], EPS)
            nc.vector.reciprocal(r2[:], nrm2[:])
            nc.vector.tensor_mul(fac[:], f1[:], r2[:])
            for c in range(C_CLS):
                nc.vector.tensor_scalar_mul(
                    v_sb[:, c, :], s_sb[:, c, :], fac[:, c : c + 1]
                )

        def build_vT():
            # in2[64ch+b, 16c+d] = v[b,c,d] for c%2==ch (classes 0-7)
            i2v = in2[:].rearrange("q (c d) -> q c d", d=D_DIM)
            nc.vector.tensor_copy(i2v[0:64, 0::2, :], v_sb[:, 0:8:2, :])
            nc.vector.tensor_copy(i2v[64:128, 1::2, :], v_sb[:, 1:8:2, :])
            # in2b cols 96:128 = classes 8,9 (rows 96:128 of vT4 after transpose)
            nc.vector.tensor_copy(in2b[0:64, 96:112], v_sb[:, 8, :])
            nc.vector.tensor_copy(in2b[64:128, 112:128], v_sb[:, 9, :])
            pv = psT.tile([128, I_CAPS], f32, tag="pt")
            nc.tensor.transpose(pv[:, 0:128], in2[:], eye_sb[:])
            nc.vector.tensor_copy(vT[:], pv[:, 0:128])
            pv4 = psT.tile([128, I_CAPS], f32, tag="pt")
            nc.tensor.transpose(pv4[:, 0:128], in2b[:], eye_sb[:])
            nc.scalar.copy(vT4[:], pv4[:, 0:128])

        def TA_phase(bts):
            for p in range(5):
                vrhs = vT[32 * p : 32 * (p + 1), :] if p < 4 else vT4[96:128, :]
                lhsW = WT if p < 4 else WTB
                row0 = 32 * p if p < 4 else 96
                Tp = itp.tile([128, K_DIM, NG, 128], bf16, tag="tp")
                for k in range(K_DIM):
                    pt = psT.tile([128, I_CAPS], f32, tag="pt")
                    for g in range(NG):
                        nc.tensor.matmul(
                            pt[:, 128 * g : 128 * (g + 1)],
                            lhsW[row0 : row0 + 32, k, 128 * g : 128 * (g + 1)],
                            vrhs,
                            start=True,
                            stop=True,
                            tile_position=(row0, 0),
                        )
                    nc.scalar.copy(
                        Tp[:, k, :, :].rearrange("r g q -> r (g q)"), pt[:]
                    )
                # P = T * u (all k at once), then tree-reduce over k
                P = itp.tile([128, K_DIM, NG, 128], bf16, tag="pp")
                nc.vector.tensor_tensor(
                    P[:].rearrange("r k g (c b) -> r k g c b", c=2),
                    Tp[:].rearrange("r k g (c b) -> r k g c b", c=2),
                    uTk[:].rearrange("r k g b -> r k g () b").to_broadcast(
                        (128, K_DIM, NG, 2, B_PER)
                    ),
                    OP.mult,
                )
                t1a = itp.tile([128, 4, NG, 128], bf16, tag="t4", bufs=1)
                nc.vector.tensor_tensor(t1a[:], P[:, 0:4], P[:, 4:8], OP.add)
                t2a = itp.tile([128, 2, NG, 128], bf16, tag="t2", bufs=1)
                nc.vector.tensor_tensor(t2a[:], t1a[:, 0:2], t1a[:, 2:4], OP.add)
                Lp = itp.tile([128, NG, 128], bf16, tag="t4", bufs=1)
                nc.vector.tensor_tensor(Lp[:], t2a[:, 0], t2a[:, 1], OP.add)
                Lv = L[:, p, :, :, :].rearrange("r g c b -> r (g c b)")
                nc.vector.tensor_tensor(
                    Lv, Lp[:].rearrange("r g q -> r (g q)"), Lv, OP.add
                )
                softmax_exp_p(p, bts[p])

        def softmax_exp_p(p, bt):
            """exp of pass p + ch-fold; call as each L_p finishes."""
            nc.scalar.activation(
                cE[:, p].rearrange("r g c b -> r (g c b)"),
                L[:, p].rearrange("r g c b -> r (g c b)"),
                AF.Exp,
            )
            nc.vector.tensor_tensor(
                bt[:], cE[:, p, :, 0, :], cE[:, p, :, 1, :], OP.add
            )

        def softmax_phase(bts):
            b0, b1, b2, b3, b4 = bts
            den = smp.tile([128, NG, B_PER], f32, tag="smd", bufs=1)
            bf32 = smp.tile([128, NG, B_PER], f32, tag="smf", bufs=1)
            nc.vector.tensor_tensor(b0[:], b0[:], b1[:], OP.add)
            nc.vector.tensor_tensor(b2[:], b2[:], b3[:], OP.add)
            nc.vector.tensor_tensor(bf32[:], b0[:], b2[:], OP.add)
            nc.vector.tensor_tensor(den[:], bf32[:], b4[:], OP.add)
            with nc.allow_low_precision(reason="softmax reciprocal to bf16 ok"):
                nc.vector.reciprocal(
                    recT[:].rearrange("r g b -> r (g b)"),
                    den[:].rearrange("r g b -> r (g b)"),
                )

        def s_phase_routed():
            uTs = itp.tile([128, K_DIM, NG, B_PER], bf16, tag="uts", bufs=1)
            nc.vector.tensor_tensor(
                uTs[:],
                uTk[:],
                recT[:].rearrange("r g b -> r () g b").to_broadcast(
                    (128, K_DIM, NG, B_PER)
                ),
                OP.mult,
            )
            for c in range(C_CLS):
                p, ch = c // 2, c % 2
                xc = itp.tile([128, K_DIM, NG, B_PER], bf16, tag="pp")
                nc.vector.tensor_tensor(
                    xc[:],
                    uTs[:],
                    cE[:, p, :, ch, :].rearrange("r g b -> r () g b").to_broadcast(
                        (128, K_DIM, NG, B_PER)
                    ),
                    OP.mult,
                )
                ps = psS.tile([64, C_CLS * D_DIM], f32, tag="ps_s")
                n = 0
                for k in range(K_DIM):
                    for g in range(NG):
                        nc.tensor.matmul(
                            ps[:, 16 * c : 16 * (c + 1)],
                            xc[:, k, g, :],
                            Wsk[:, k, g, c, :],
                            start=(n == 0),
                            stop=(n == K_DIM * NG - 1),
                        )
                        n += 1
                nc.scalar.copy(s_sb[:, c, :], ps[:, 16 * c : 16 * (c + 1)])

        # ---------------- main flow ----------------
        import os
        kstage = int(os.environ.get("KSTAGE", "99"))
        s_phase_s0()
        squash()
        if kstage >= 1:
            for j in range(2):
                build_vT()
                bts = []
                for i in range(5):
                    bti = smp.tile(
                        [128, NG, B_PER], bf16, tag=f"sm{i}", bufs=1,
                        name=f"bt{i}",
                    )
                    bts.append(bti)
                TA_phase(bts)
                if kstage == 1 + 3 * j:
                    break
                softmax_phase(bts)
                if kstage == 2 + 3 * j:
                    break
                s_phase_routed()
                squash()
                if kstage == 3 + 3 * j:
                    break
        nc.sync.dma_start(v_out[:], v_sb[:])

        for pool in (smp, itp, setup, psT, psS, perm):
            try:
                pool.release()
            except Exception:
                pass

    nc.compile()
    return nc


def _consts():
    return {
        "eye128": np.eye(128, dtype=np.float32),
    }


def get_nc():
    if "nc" not in _CACHE:
        _CACHE["nc"] = _build()
    return _CACHE["nc"]


def kernel(u: np.ndarray, W: np.ndarray) -> np.ndarray:
    from concourse.bass_utils import run_bass_kernel_spmd

    nc = get_nc()
    consts = _consts()
    w_full = np.ascontiguousarray(W[0]).astype(np.float32)
    in_maps = []
    for core in range(N_CORES):
        sh = np.ascontiguousarray(u[core * B_PER : (core + 1) * B_PER]).astype(
            np.float32
        )
        in_maps.append({"u": sh, "w": w_full, **consts})
    res = run_bass_kernel_spmd(nc, in_maps, list(range(N_CORES)))
    out = np.concatenate([res.results[i]["v"] for i in range(N_CORES)], axis=0)
    return out.astype(np.float32)
